# revision 31
# baseline (speedup 1.0000x reference)
"""Trainium2 Bass kernel for a 2D NeRF-style MLP.

Network (per point):
    enc = [cos(u), cos(v), sin(u), sin(v)]            # [4]
    h0  = relu(enc @ W_in + b_in)                     # [256]
    h1  = relu(h0 @ W_h0 + b_h0)                      # [256]
    h2  = relu(h1 @ W_h1 + b_h1)                      # [256]
    out = sigmoid(h2 @ W_out + b_out)                 # [3]

Strategy: pure data parallel over 8 NeuronCores (65536 points each).
On-chip dataflow is feature-major (activations live as h.T with features
on SBUF partitions, batch in the free dim) so the small MLP weights are
the stationary matmul operand. Matmuls run in float32r (full PE rate at
free-dim 512).

The [4 x batch] encoded input for layer 1 is produced without any
strided HBM traffic: uv is loaded contiguously as U[128, 1024]
(partition p holds points 512p..512p+511, coords interleaved), cos/sin
are computed at full 128-lane ACT occupancy, then a DVE per-32x32-block
stream transpose moves features onto partitions. Each 32-wide free
block of the transpose input holds the 4 features of one point-column
group padded with 28 zeros, so after the block transpose the features
land at 32-aligned partition bases - legal matmul operand bases with
K=32 (the 28 zero rows contribute nothing). Layer-1 weights are
replicated at each of the 4 row-group bases.

The device writes out.T as [3, 65536] in tile-permuted column order
(all DMAs fully contiguous); the host inverts the permutation when
assembling the full [N, 3] result.
"""

import math

import ml_dtypes
import numpy as np

import concourse.bass as bass
import concourse.bass_utils as bass_utils
import concourse.mybir as mybir
import concourse.tile as tile
from concourse import bacc

MODE = "bf16"  # "f32r" | "bf16"
SIG_BATCH = False  # col-tiled psum L4 miscomputes on HW; no speedup anyway
N_CORES = 8
N_TOTAL = 524288
N_PER = N_TOTAL // N_CORES  # 65536 points per core
C = 256  # hidden width
NT = 32  # t-tiles per core; each covers 2048 points
BX = 4  # zero-padded transpose-staging slots

F32 = mybir.dt.float32
F32R = mybir.dt.float32r
BF16 = mybir.dt.bfloat16


def _emit(
    tc,
    nc,
    uv,
    w_in,
    b_in,
    w_h0,
    b_h0,
    w_h1,
    b_h1,
    w_out,
    b_out,
    out,
    nt=NT,
    reps=1,
    mode="f32r",
):
    MMDT = BF16 if mode == "bf16" else F32R
    Relu = mybir.ActivationFunctionType.Relu
    Sin = mybir.ActivationFunctionType.Sin
    Sigmoid = mybir.ActivationFunctionType.Sigmoid
    add = mybir.AluOpType.add
    mx = mybir.AluOpType.max

    with (
        tc.tile_pool(name="wpool", bufs=1) as wpool,
        tc.tile_pool(name="upool", bufs=1) as upool,
        tc.tile_pool(name="xpool", bufs=1) as xpool,
        tc.tile_pool(name="rpool", bufs=3) as rpool,
        tc.tile_pool(name="hpool", bufs=6) as hpool,
        tc.tile_pool(name="opool", bufs=2) as opool,
        tc.tile_pool(name="pspool", bufs=6, space=bass.MemorySpace.PSUM) as pspool,
        tc.tile_pool(name="ps3pool", bufs=2, space=bass.MemorySpace.PSUM) as ps3pool,
    ):
        # ---- input load + trig first so the first tile's compute
        # unblocks before the weight DMAs drain; the first 128 columns are
        # computed separately so t=0..3 unblock even sooner ----
        halfpi = wpool.tile([128, 1], F32, tag="halfpi")
        nc.gpsimd.memset(halfpi[:], math.pi / 2)
        zerob = wpool.tile([128, 1], F32, tag="zerob")
        nc.gpsimd.memset(zerob[:], 0.0)
        u = upool.tile([128, 1024], F32, tag="u")
        nc.sync.dma_start(u[:, 0:128], uv.rearrange("(p j) c -> p (j c)", p=128)[:, 0:128])
        nc.sync.dma_start(u[:, 128:1024], uv.rearrange("(p j) c -> p (j c)", p=128)[:, 128:1024])
        ucos = upool.tile([128, 1024], F32, tag="ucos")
        usin = upool.tile([128, 1024], F32, tag="usin")
        nc.scalar.activation(ucos[:, 0:128], u[:, 0:128], Sin, bias=halfpi[:])
        nc.scalar.activation(usin[:, 0:128], u[:, 0:128], Sin, bias=zerob[:])
        nc.scalar.activation(ucos[:, 128:1024], u[:, 128:1024], Sin, bias=halfpi[:])
        nc.scalar.activation(usin[:, 128:1024], u[:, 128:1024], Sin, bias=zerob[:])

        # ---- weights / biases (persistent) ----
        # Layer-1 weights replicated at each 32-partition row-group base,
        # rows 4..31 of each group zeroed (they multiply the zero-padded
        # rows of the transposed encoding).
        wpad = wpool.tile([128, C], MMDT, tag="wpad")
        if mode == "bf16":
            nc.gpsimd.memset(wpad[:], 0.0)
        else:
            nc.gpsimd.memset(wpad[:].bitcast(F32), 0.0)
        for a in range(4):
            nc.sync.dma_start(wpad[32 * a : 32 * a + 4, :], w_in)

        # Hidden weights as two K-tiles side by side: w[p, kt*C + m] =
        # W[kt*128 + p, m].
        wh0 = wpool.tile([128, 2 * C], MMDT, tag="wh0")
        nc.sync.dma_start(
            wh0.rearrange("p (kt m) -> p kt m", kt=2),
            w_h0.rearrange("(kt p) m -> p kt m", kt=2),
        )
        wh1 = wpool.tile([128, 2 * C], MMDT, tag="wh1")
        nc.sync.dma_start(
            wh1.rearrange("p (kt m) -> p kt m", kt=2),
            w_h1.rearrange("(kt p) m -> p kt m", kt=2),
        )
        if mode == "bf16" and SIG_BATCH:
            # W_out padded to M=32 per K-tile (columns 3..31 zero): the four
            # a-groups' outputs land in one shared PSUM bank at partition
            # bases 0/32/64/96 (col tiling - legal for bf16 matmuls) and one
            # sigmoid covers all four.
            wout = wpool.tile([128, 64], MMDT, tag="wout")
            nc.gpsimd.memset(wout[:], 0.0)
            nc.sync.dma_start(
                wout.rearrange("p (kt m) -> p kt m", kt=2)[:, :, 0:3],
                w_out.rearrange("(kt p) m -> p kt m", kt=2),
            )
        else:
            wout = wpool.tile([128, 6], MMDT, tag="wout")
            nc.sync.dma_start(
                wout.rearrange("p (kt m) -> p kt m", kt=2),
                w_out.rearrange("(kt p) m -> p kt m", kt=2),
            )

        bin_sb = wpool.tile([128, 2], F32, tag="bin")
        nc.gpsimd.dma_start(bin_sb[:], b_in.rearrange("(mh p) -> p mh", mh=2))
        bh0_sb = wpool.tile([128, 2], F32, tag="bh0")
        nc.gpsimd.dma_start(bh0_sb[:], b_h0.rearrange("(mh p) -> p mh", mh=2))
        bh1_sb = wpool.tile([128, 2], F32, tag="bh1")
        nc.gpsimd.dma_start(bh1_sb[:], b_h1.rearrange("(mh p) -> p mh", mh=2))
        if mode == "bf16" and SIG_BATCH:
            # b_out replicated at partitions 32a + m (pad partitions bias 0).
            bout_sb = wpool.tile([128, 1], F32, tag="bout")
            nc.gpsimd.memset(bout_sb[:], 0.0)
            for a in range(4):
                nc.gpsimd.dma_start(
                    bout_sb[32 * a : 32 * a + 3, :],
                    b_out.rearrange("(c o) -> c o", o=1),
                )
        else:
            bout_sb = wpool.tile([3, 1], F32, tag="bout")
            nc.sync.dma_start(bout_sb[:], b_out.rearrange("(c o) -> c o", o=1))

        # ---- zero-padded transpose-staging slots (pad columns stay zero
        # because every iteration rewrites only columns 0..3 of each
        # 32-wide block) ----
        xtiles = []
        for i in range(BX):
            x = xpool.tile([128, 512], F32 if mode == "f32r" else BF16, tag=f"x{i}", name=f"xstage{i}")
            nc.gpsimd.memset(x[:], 0.0)
            xtiles.append(x)

        for t in [tt for _ in range(reps) for tt in range(nt)]:
            x = xtiles[t % BX]
            # x[p, 32b + c]     = cos(uv[512p + 16t + b, c])
            # x[p, 32b + 2 + c] = sin(uv[512p + 16t + b, c])
            nc.vector.tensor_copy(
                x.rearrange("p (b q) -> p b q", q=32)[:, :, 0:2],
                ucos[:, 32 * t : 32 * t + 32].rearrange("p (b c) -> p b c", c=2),
            )
            nc.vector.tensor_copy(
                x.rearrange("p (b q) -> p b q", q=32)[:, :, 2:4],
                usin[:, 32 * t : 32 * t + 32].rearrange("p (b c) -> p b c", c=2),
            )
            # Per-32x32-block transpose: r[32a + f, 32b + pl] = x[32a + pl, 32b + f]
            # (StreamTranspose has no fp32r path, so transpose in f32 and
            # retype via a byte-moving DMA - the BIR verifier only requires
            # the direct producer of an fp32r matmul operand to carry the
            # f32r dtype, and the PE rounds internally on read.)
            r = rpool.tile([128, 512], MMDT, tag="r", name="renc")
            if mode == "bf16":
                nc.vector.transpose(r[:], x[:])
            else:
                rraw = rpool.tile([128, 512], F32, tag="rraw", name="rraw")
                nc.vector.transpose(rraw[:], x[:])
                nc.sync.dma_start(r[:], rraw[:].bitcast(F32R))

            if mode == "bf16" and SIG_BATCH:
                ps4 = ps3pool.tile([128, 512], F32, tag="ps4", name="ps4")
            else:
                ot = opool.tile([3, 2048], F32, tag="ot", name="otile")
            for pair in ((0, 1), (2, 3)):
                # Layer-staged over two independent point-streams so the PE
                # fills one stream's epilogue latency with the other's
                # matmuls.
                ps0 = {}
                for a in pair:
                    rh = r[32 * a : 32 * a + 32, :]
                    pa = pspool.tile([128, 512], F32, tag="ps", name="ps0a")
                    pb = pspool.tile([128, 512], F32, tag="ps", name="ps0b")
                    nc.tensor.matmul(
                        pa[:],
                        wpad[32 * a : 32 * a + 32, 0:128],
                        rh,
                        tile_position=(32 * a, 0),
                    )
                    nc.tensor.matmul(
                        pb[:],
                        wpad[32 * a : 32 * a + 32, 128:256],
                        rh,
                        tile_position=(32 * a, 0),
                    )
                    ps0[a] = (pa, pb)

                def epilogue(ps_pair, bias, idx, name):
                    # One half on ACT, one on DVE - the two halves must run
                    # in parallel or the layer chain's latency grows.
                    h = hpool.tile([128, 1024], MMDT, tag="h", name=name)
                    pa, pb = ps_pair
                    if idx % 2 == 0:
                        nc.scalar.activation(
                            h[:, 0:512], pa[:], Relu, bias=bias[:, 0:1]
                        )
                        nc.vector.tensor_scalar(
                            h[:, 512:1024], pb[:], bias[:, 1:2], 0.0, add, mx
                        )
                    else:
                        nc.vector.tensor_scalar(
                            h[:, 0:512], pa[:], bias[:, 0:1], 0.0, add, mx
                        )
                        nc.scalar.activation(
                            h[:, 512:1024], pb[:], Relu, bias=bias[:, 1:2]
                        )
                    return h

                def hidden_mms(w, h_prev):
                    pa = pspool.tile([128, 512], F32, tag="ps", name="psha")
                    pb = pspool.tile([128, 512], F32, tag="ps", name="pshb")
                    for kt in range(2):
                        nc.tensor.matmul(
                            pa[:],
                            w[:, kt * C : kt * C + 128],
                            h_prev[:, kt * 512 : (kt + 1) * 512],
                            start=(kt == 0),
                            stop=(kt == 1),
                        )
                    for kt in range(2):
                        nc.tensor.matmul(
                            pb[:],
                            w[:, kt * C + 128 : kt * C + 256],
                            h_prev[:, kt * 512 : (kt + 1) * 512],
                            start=(kt == 0),
                            stop=(kt == 1),
                        )
                    return (pa, pb)

                h0 = {a: epilogue(ps0[a], bin_sb, i, "h0") for i, a in enumerate(pair)}
                ps1 = {a: hidden_mms(wh0, h0[a]) for a in pair}
                h1 = {a: epilogue(ps1[a], bh0_sb, i + 1, "h1") for i, a in enumerate(pair)}
                ps2 = {a: hidden_mms(wh1, h1[a]) for a in pair}
                h2 = {a: epilogue(ps2[a], bh1_sb, i, "h2") for i, a in enumerate(pair)}

                for a in pair:
                    if mode == "bf16" and SIG_BATCH:
                        nc.tensor.matmul(
                            ps4[32 * a : 32 * a + 32, :],
                            wout[:, 0:32],
                            h2[a][:, 0:512],
                            start=True,
                            stop=False,
                            tile_position=(0, 32 * a),
                        )
                        nc.tensor.matmul(
                            ps4[32 * a : 32 * a + 32, :],
                            wout[:, 32:64],
                            h2[a][:, 512:1024],
                            start=False,
                            stop=True,
                            tile_position=(0, 32 * a),
                        )
                    else:
                        ps3 = ps3pool.tile([3, 512], F32, tag="ps3", name="ps3")
                        nc.tensor.matmul(
                            ps3[:],
                            wout[:, 0:3],
                            h2[a][:, 0:512],
                            start=True,
                            stop=False,
                        )
                        nc.tensor.matmul(
                            ps3[:],
                            wout[:, 3:6],
                            h2[a][:, 512:1024],
                            start=False,
                            stop=True,
                        )
                        nc.scalar.activation(
                            ot[:, 512 * a : 512 * (a + 1)],
                            ps3[:],
                            Sigmoid,
                            bias=bout_sb[:, 0:1],
                        )

            if mode == "bf16" and SIG_BATCH:
                osb = opool.tile([128, 512], F32, tag="ot", name="osb")
                nc.scalar.activation(osb[:], ps4[:], Sigmoid, bias=bout_sb[:, 0:1])
                nc.sync.dma_start(
                    out[:, 2048 * t : 2048 * (t + 1)].rearrange(
                        "c (a n) -> c a n", a=4
                    ),
                    osb.rearrange("(a q) n -> q a n", a=4)[0:3],
                )
            else:
                nc.sync.dma_start(out[:, 2048 * t : 2048 * (t + 1)], ot[:])


_prog_cache = {}


def _program(nt=NT, reps=1, mode=MODE):
    key = (nt, reps, mode, SIG_BATCH)
    if key in _prog_cache:
        return _prog_cache[key]
    nc = bacc.Bacc(
        "TRN2", target_bir_lowering=False, debug=False, num_devices=N_CORES
    )
    wdt = BF16 if mode == "bf16" else F32R
    uv_d = nc.dram_tensor("uv", [N_PER, 2], F32, kind="ExternalInput")
    w_in_d = nc.dram_tensor("w_in", [4, C], wdt, kind="ExternalInput")
    b_in_d = nc.dram_tensor("b_in", [C], F32, kind="ExternalInput")
    w_h0_d = nc.dram_tensor("w_h0", [C, C], wdt, kind="ExternalInput")
    b_h0_d = nc.dram_tensor("b_h0", [C], F32, kind="ExternalInput")
    w_h1_d = nc.dram_tensor("w_h1", [C, C], wdt, kind="ExternalInput")
    b_h1_d = nc.dram_tensor("b_h1", [C], F32, kind="ExternalInput")
    w_out_d = nc.dram_tensor("w_out", [C, 3], wdt, kind="ExternalInput")
    b_out_d = nc.dram_tensor("b_out", [3], F32, kind="ExternalInput")
    out_d = nc.dram_tensor("out_t", [3, N_PER], F32, kind="ExternalOutput")
    with tile.TileContext(nc) as tc:
        _emit(
            tc,
            nc,
            uv_d.ap(),
            w_in_d.ap(),
            b_in_d.ap(),
            w_h0_d.ap(),
            b_h0_d.ap(),
            w_h1_d.ap(),
            b_h1_d.ap(),
            w_out_d.ap(),
            b_out_d.ap(),
            out_d.ap(),
            nt=nt,
            reps=reps,
            mode=mode,
        )
    nc.compile()
    _prog_cache[key] = nc
    return nc


def _col_perm():
    """Point index for each device-output column s (per core).

    Device column s = 2048t + 512a + 32b + pl maps to point
    n = 512*(32a + pl) + 16t + b.
    """
    s = np.arange(N_PER)
    t = s >> 11
    a = (s >> 9) & 3
    b = (s >> 5) & 15
    pl = s & 31
    return 512 * (32 * a + pl) + 16 * t + b


def kernel(uv, W_in, b_in, W_h0, b_h0, W_h1, b_h1, W_out, b_out):
    nc = _program()
    wdt = ml_dtypes.bfloat16 if MODE == "bf16" else np.float32
    weights = {
        "w_in": np.ascontiguousarray(W_in, wdt),
        "b_in": np.ascontiguousarray(b_in, np.float32),
        "w_h0": np.ascontiguousarray(W_h0, wdt),
        "b_h0": np.ascontiguousarray(b_h0, np.float32),
        "w_h1": np.ascontiguousarray(W_h1, wdt),
        "b_h1": np.ascontiguousarray(b_h1, np.float32),
        "w_out": np.ascontiguousarray(W_out, wdt),
        "b_out": np.ascontiguousarray(b_out, np.float32),
    }
    uv = np.ascontiguousarray(uv, np.float32)
    in_maps = [
        {"uv": uv[c * N_PER : (c + 1) * N_PER], **weights} for c in range(N_CORES)
    ]
    res = bass_utils.run_bass_kernel_spmd(nc, in_maps, core_ids=list(range(N_CORES)))

    perm = _col_perm()
    full = np.empty((N_TOTAL, 3), np.float32)
    for c in range(N_CORES):
        block = full[c * N_PER : (c + 1) * N_PER]
        block[perm] = res.results[c]["out_t"].T
    return full


# revision 38
# speedup vs baseline: 1.1270x; 1.1270x over previous
"""Trainium2 Bass kernel for a 2D NeRF-style MLP.

Network (per point):
    enc = [cos(u), cos(v), sin(u), sin(v)]            # [4]
    h0  = relu(enc @ W_in + b_in)                     # [256]
    h1  = relu(h0 @ W_h0 + b_h0)                      # [256]
    h2  = relu(h1 @ W_h1 + b_h1)                      # [256]
    out = sigmoid(h2 @ W_out + b_out)                 # [3]

Strategy: pure data parallel over 8 NeuronCores (65536 points each).
On-chip dataflow is feature-major (activations live as h.T with features
on SBUF partitions, batch of 512 points in the matmul free dim) so the
small MLP weights are the stationary matmul operand. Matmul operands are
bf16 (HW-measured ~20% faster than float32r - fp32r's fused 4-byte
weight load serializes against the stream - and max abs output error is
only ~1.4e-4); PSUM accumulation, biases, trig and sigmoid stay fp32.

The [4 x batch] encoded input for layer 1 is produced without any
strided HBM traffic: uv is loaded contiguously as U[128, 1024]
(partition p holds points 512p..512p+511, coords interleaved), cos/sin
are computed at full 128-lane ACT occupancy, then a DVE per-32x32-block
stream transpose moves features onto partitions. Each 32-wide free
block of the transpose input holds the 4 features of one point-column
group padded with 28 zeros, so after the block transpose the features
land at 32-aligned partition bases - legal matmul operand bases with
K=32 (the 28 zero rows contribute nothing). Layer-1 weights are
replicated at each of the 4 row-group bases.

Per 2048-point tile, the four 512-point streams are emitted
layer-staged so the PE fills one stream's relu/bias epilogue latency
(ACT and DVE, one M-half each, in parallel) with another stream's
matmuls. The device writes out.T as [3, 65536] in tile-permuted column
order (all DMAs fully contiguous); the host inverts the permutation
when assembling the full [N, 3] result.
"""

import math

import ml_dtypes
import numpy as np

import concourse.bass as bass
import concourse.bass_utils as bass_utils
import concourse.mybir as mybir
import concourse.tile as tile
from concourse import bacc

MODE = "bf16"  # "f32r" | "bf16"
SIG_BATCH = False  # col-tiled psum L4 miscomputes on HW; no speedup anyway
N_CORES = 8
N_TOTAL = 524288
N_PER = N_TOTAL // N_CORES  # 65536 points per core
C = 256  # hidden width
NT = 32  # t-tiles per core; each covers 2048 points
BX = 4  # zero-padded transpose-staging slots

F32 = mybir.dt.float32
F32R = mybir.dt.float32r
BF16 = mybir.dt.bfloat16


def _emit(
    tc,
    nc,
    uv,
    w_in,
    b_in,
    w_h0,
    b_h0,
    w_h1,
    b_h1,
    w_out,
    b_out,
    out,
    nt=NT,
    reps=1,
    mode="f32r",
):
    MMDT = BF16 if mode == "bf16" else F32R
    Relu = mybir.ActivationFunctionType.Relu
    Sin = mybir.ActivationFunctionType.Sin
    Sigmoid = mybir.ActivationFunctionType.Sigmoid
    add = mybir.AluOpType.add
    mx = mybir.AluOpType.max

    with (
        tc.tile_pool(name="wpool", bufs=1) as wpool,
        tc.tile_pool(name="upool", bufs=1) as upool,
        tc.tile_pool(name="xpool", bufs=1) as xpool,
        tc.tile_pool(name="rpool", bufs=3) as rpool,
        tc.tile_pool(name="hpool", bufs=6) as hpool,
        tc.tile_pool(name="opool", bufs=2) as opool,
        tc.tile_pool(name="pspool", bufs=6, space=bass.MemorySpace.PSUM) as pspool,
        tc.tile_pool(name="ps3pool", bufs=2, space=bass.MemorySpace.PSUM) as ps3pool,
    ):
        # ---- input load + trig first so the first tile's compute
        # unblocks before the weight DMAs drain; the first 128 columns are
        # computed separately so t=0..3 unblock even sooner ----
        halfpi = wpool.tile([128, 1], F32, tag="halfpi")
        nc.gpsimd.memset(halfpi[:], math.pi / 2)
        zerob = wpool.tile([128, 1], F32, tag="zerob")
        nc.gpsimd.memset(zerob[:], 0.0)
        u = upool.tile([128, 1024], F32, tag="u")
        nc.sync.dma_start(u[:, 0:128], uv.rearrange("(p j) c -> p (j c)", p=128)[:, 0:128])
        nc.sync.dma_start(u[:, 128:1024], uv.rearrange("(p j) c -> p (j c)", p=128)[:, 128:1024])
        ucos = upool.tile([128, 1024], F32, tag="ucos")
        usin = upool.tile([128, 1024], F32, tag="usin")
        nc.scalar.activation(ucos[:, 0:128], u[:, 0:128], Sin, bias=halfpi[:])
        nc.scalar.activation(usin[:, 0:128], u[:, 0:128], Sin, bias=zerob[:])
        nc.scalar.activation(ucos[:, 128:1024], u[:, 128:1024], Sin, bias=halfpi[:])
        nc.scalar.activation(usin[:, 128:1024], u[:, 128:1024], Sin, bias=zerob[:])

        # ---- weights / biases (persistent) ----
        # Layer-1 weights replicated at each 32-partition row-group base,
        # rows 4..31 of each group zeroed (they multiply the zero-padded
        # rows of the transposed encoding).
        wpad = wpool.tile([128, C], MMDT, tag="wpad")
        if mode == "bf16":
            nc.gpsimd.memset(wpad[:], 0.0)
        else:
            nc.gpsimd.memset(wpad[:].bitcast(F32), 0.0)
        for a in range(4):
            nc.sync.dma_start(wpad[32 * a : 32 * a + 4, :], w_in)

        # Hidden weights as two K-tiles side by side: w[p, kt*C + m] =
        # W[kt*128 + p, m].
        wh0 = wpool.tile([128, 2 * C], MMDT, tag="wh0")
        nc.sync.dma_start(
            wh0.rearrange("p (kt m) -> p kt m", kt=2),
            w_h0.rearrange("(kt p) m -> p kt m", kt=2),
        )
        wh1 = wpool.tile([128, 2 * C], MMDT, tag="wh1")
        nc.sync.dma_start(
            wh1.rearrange("p (kt m) -> p kt m", kt=2),
            w_h1.rearrange("(kt p) m -> p kt m", kt=2),
        )
        if mode == "bf16" and SIG_BATCH:
            # W_out padded to M=32 per K-tile (columns 3..31 zero): the four
            # a-groups' outputs land in one shared PSUM bank at partition
            # bases 0/32/64/96 (col tiling - legal for bf16 matmuls) and one
            # sigmoid covers all four.
            wout = wpool.tile([128, 64], MMDT, tag="wout")
            nc.gpsimd.memset(wout[:], 0.0)
            nc.sync.dma_start(
                wout.rearrange("p (kt m) -> p kt m", kt=2)[:, :, 0:3],
                w_out.rearrange("(kt p) m -> p kt m", kt=2),
            )
        else:
            wout = wpool.tile([128, 6], MMDT, tag="wout")
            nc.sync.dma_start(
                wout.rearrange("p (kt m) -> p kt m", kt=2),
                w_out.rearrange("(kt p) m -> p kt m", kt=2),
            )

        bin_sb = wpool.tile([128, 2], F32, tag="bin")
        nc.gpsimd.dma_start(bin_sb[:], b_in.rearrange("(mh p) -> p mh", mh=2))
        bh0_sb = wpool.tile([128, 2], F32, tag="bh0")
        nc.gpsimd.dma_start(bh0_sb[:], b_h0.rearrange("(mh p) -> p mh", mh=2))
        bh1_sb = wpool.tile([128, 2], F32, tag="bh1")
        nc.gpsimd.dma_start(bh1_sb[:], b_h1.rearrange("(mh p) -> p mh", mh=2))
        if mode == "bf16" and SIG_BATCH:
            # b_out replicated at partitions 32a + m (pad partitions bias 0).
            bout_sb = wpool.tile([128, 1], F32, tag="bout")
            nc.gpsimd.memset(bout_sb[:], 0.0)
            for a in range(4):
                nc.gpsimd.dma_start(
                    bout_sb[32 * a : 32 * a + 3, :],
                    b_out.rearrange("(c o) -> c o", o=1),
                )
        else:
            bout_sb = wpool.tile([3, 1], F32, tag="bout")
            nc.sync.dma_start(bout_sb[:], b_out.rearrange("(c o) -> c o", o=1))

        # ---- zero-padded transpose-staging slots (pad columns stay zero
        # because every iteration rewrites only columns 0..3 of each
        # 32-wide block) ----
        xtiles = []
        for i in range(BX):
            x = xpool.tile([128, 512], F32 if mode == "f32r" else BF16, tag=f"x{i}", name=f"xstage{i}")
            nc.gpsimd.memset(x[:], 0.0)
            xtiles.append(x)

        for t in [tt for _ in range(reps) for tt in range(nt)]:
            x = xtiles[t % BX]
            # x[p, 32b + c]     = cos(uv[512p + 16t + b, c])
            # x[p, 32b + 2 + c] = sin(uv[512p + 16t + b, c])
            nc.vector.tensor_copy(
                x.rearrange("p (b q) -> p b q", q=32)[:, :, 0:2],
                ucos[:, 32 * t : 32 * t + 32].rearrange("p (b c) -> p b c", c=2),
            )
            nc.vector.tensor_copy(
                x.rearrange("p (b q) -> p b q", q=32)[:, :, 2:4],
                usin[:, 32 * t : 32 * t + 32].rearrange("p (b c) -> p b c", c=2),
            )
            # Per-32x32-block transpose: r[32a + f, 32b + pl] = x[32a + pl, 32b + f]
            # (StreamTranspose has no fp32r path, so transpose in f32 and
            # retype via a byte-moving DMA - the BIR verifier only requires
            # the direct producer of an fp32r matmul operand to carry the
            # f32r dtype, and the PE rounds internally on read.)
            r = rpool.tile([128, 512], MMDT, tag="r", name="renc")
            if mode == "bf16":
                nc.vector.transpose(r[:], x[:])
            else:
                rraw = rpool.tile([128, 512], F32, tag="rraw", name="rraw")
                nc.vector.transpose(rraw[:], x[:])
                nc.sync.dma_start(r[:], rraw[:].bitcast(F32R))

            ot = opool.tile([3, 2048], F32, tag="ot", name="otile")
            for pair in ((0, 1, 2, 3),):
                ps0 = {}
                for a in pair:
                    rh = r[32 * a : 32 * a + 32, :]
                    pa = pspool.tile([128, 512], F32, tag="ps", name="ps0a")
                    pb = pspool.tile([128, 512], F32, tag="ps", name="ps0b")
                    nc.tensor.matmul(
                        pa[:],
                        wpad[32 * a : 32 * a + 32, 0:128],
                        rh,
                        tile_position=(32 * a, 0),
                    )
                    nc.tensor.matmul(
                        pb[:],
                        wpad[32 * a : 32 * a + 32, 128:256],
                        rh,
                        tile_position=(32 * a, 0),
                    )
                    ps0[a] = (pa, pb)

                def epilogue(ps_pair, bias, idx, name):
                    h = hpool.tile([128, 1024], MMDT, tag="h", name=name)
                    pa, pb = ps_pair
                    if idx % 2 == 0:
                        nc.scalar.activation(
                            h[:, 0:512], pa[:], Relu, bias=bias[:, 0:1]
                        )
                        nc.vector.tensor_scalar(
                            h[:, 512:1024], pb[:], bias[:, 1:2], 0.0, add, mx
                        )
                    else:
                        nc.vector.tensor_scalar(
                            h[:, 0:512], pa[:], bias[:, 0:1], 0.0, add, mx
                        )
                        nc.scalar.activation(
                            h[:, 512:1024], pb[:], Relu, bias=bias[:, 1:2]
                        )
                    return h

                def hidden_mms(w, h_prev):
                    pa = pspool.tile([128, 512], F32, tag="ps", name="psha")
                    pb = pspool.tile([128, 512], F32, tag="ps", name="pshb")
                    for kt in range(2):
                        nc.tensor.matmul(
                            pa[:],
                            w[:, kt * C : kt * C + 128],
                            h_prev[:, kt * 512 : (kt + 1) * 512],
                            start=(kt == 0),
                            stop=(kt == 1),
                        )
                    for kt in range(2):
                        nc.tensor.matmul(
                            pb[:],
                            w[:, kt * C + 128 : kt * C + 256],
                            h_prev[:, kt * 512 : (kt + 1) * 512],
                            start=(kt == 0),
                            stop=(kt == 1),
                        )
                    return (pa, pb)

                h0 = {a: epilogue(ps0[a], bin_sb, i, "h0") for i, a in enumerate(pair)}
                ps1 = {a: hidden_mms(wh0, h0[a]) for a in pair}
                h1 = {a: epilogue(ps1[a], bh0_sb, i + 1, "h1") for i, a in enumerate(pair)}
                ps2 = {a: hidden_mms(wh1, h1[a]) for a in pair}
                h2 = {a: epilogue(ps2[a], bh1_sb, i, "h2") for i, a in enumerate(pair)}

                for a in pair:
                    ps3 = ps3pool.tile([3, 512], F32, tag="ps3", name="ps3")
                    nc.tensor.matmul(
                        ps3[:],
                        wout[:, 0:3],
                        h2[a][:, 0:512],
                        start=True,
                        stop=False,
                    )
                    nc.tensor.matmul(
                        ps3[:],
                        wout[:, 3:6],
                        h2[a][:, 512:1024],
                        start=False,
                        stop=True,
                    )
                    nc.scalar.activation(
                        ot[:, 512 * a : 512 * (a + 1)],
                        ps3[:],
                        Sigmoid,
                        bias=bout_sb[:, 0:1],
                    )

            nc.sync.dma_start(out[:, 2048 * t : 2048 * (t + 1)], ot[:])


_prog_cache = {}


def _program(nt=NT, reps=1, mode=MODE):
    key = (nt, reps, mode, SIG_BATCH)
    if key in _prog_cache:
        return _prog_cache[key]
    nc = bacc.Bacc(
        "TRN2", target_bir_lowering=False, debug=False, num_devices=N_CORES
    )
    wdt = BF16 if mode == "bf16" else F32R
    uv_d = nc.dram_tensor("uv", [N_PER, 2], F32, kind="ExternalInput")
    w_in_d = nc.dram_tensor("w_in", [4, C], wdt, kind="ExternalInput")
    b_in_d = nc.dram_tensor("b_in", [C], F32, kind="ExternalInput")
    w_h0_d = nc.dram_tensor("w_h0", [C, C], wdt, kind="ExternalInput")
    b_h0_d = nc.dram_tensor("b_h0", [C], F32, kind="ExternalInput")
    w_h1_d = nc.dram_tensor("w_h1", [C, C], wdt, kind="ExternalInput")
    b_h1_d = nc.dram_tensor("b_h1", [C], F32, kind="ExternalInput")
    w_out_d = nc.dram_tensor("w_out", [C, 3], wdt, kind="ExternalInput")
    b_out_d = nc.dram_tensor("b_out", [3], F32, kind="ExternalInput")
    out_d = nc.dram_tensor("out_t", [3, N_PER], F32, kind="ExternalOutput")
    with tile.TileContext(nc) as tc:
        _emit(
            tc,
            nc,
            uv_d.ap(),
            w_in_d.ap(),
            b_in_d.ap(),
            w_h0_d.ap(),
            b_h0_d.ap(),
            w_h1_d.ap(),
            b_h1_d.ap(),
            w_out_d.ap(),
            b_out_d.ap(),
            out_d.ap(),
            nt=nt,
            reps=reps,
            mode=mode,
        )
    nc.compile()
    _prog_cache[key] = nc
    return nc


def _col_perm():
    """Point index for each device-output column s (per core).

    Device column s = 2048t + 512a + 32b + pl maps to point
    n = 512*(32a + pl) + 16t + b.
    """
    s = np.arange(N_PER)
    t = s >> 11
    a = (s >> 9) & 3
    b = (s >> 5) & 15
    pl = s & 31
    return 512 * (32 * a + pl) + 16 * t + b


def kernel(uv, W_in, b_in, W_h0, b_h0, W_h1, b_h1, W_out, b_out):
    nc = _program()
    wdt = ml_dtypes.bfloat16 if MODE == "bf16" else np.float32
    weights = {
        "w_in": np.ascontiguousarray(W_in, wdt),
        "b_in": np.ascontiguousarray(b_in, np.float32),
        "w_h0": np.ascontiguousarray(W_h0, wdt),
        "b_h0": np.ascontiguousarray(b_h0, np.float32),
        "w_h1": np.ascontiguousarray(W_h1, wdt),
        "b_h1": np.ascontiguousarray(b_h1, np.float32),
        "w_out": np.ascontiguousarray(W_out, wdt),
        "b_out": np.ascontiguousarray(b_out, np.float32),
    }
    uv = np.ascontiguousarray(uv, np.float32)
    in_maps = [
        {"uv": uv[c * N_PER : (c + 1) * N_PER], **weights} for c in range(N_CORES)
    ]
    res = bass_utils.run_bass_kernel_spmd(nc, in_maps, core_ids=list(range(N_CORES)))

    perm = _col_perm()
    full = np.empty((N_TOTAL, 3), np.float32)
    for c in range(N_CORES):
        block = full[c * N_PER : (c + 1) * N_PER]
        block[perm] = res.results[c]["out_t"].T
    return full
